# revision 5
# baseline (speedup 1.0000x reference)
"""ChannelGuidedAttn Trainium2 kernel.

Reference computation (per batch b):
    q  = x_pre[b]      reshaped (C, N),  C=512, N=H*W=4096
    kv = x_training[b] reshaped (C, N)
    energy[c,d] = <q[c,:], kv[d,:]>                      (C x C)
    att = softmax(max_d(energy) - energy, axis=-1)       == softmax(-energy)
        = exp(min_d(energy) - energy) / sum
    out = att @ kv  -> (C, H, W);  final softmax over W

Sharding: data-parallel over batch B=16 across 8 cores (2 batches/core).

Design (v2): all transposes/casts are done on the HOST during sharding; the
device receives three pre-laid-out fp16 tensors per batch:
  - ktT[p, j, d] = kv[d, j*128+p]   (gemm1 moving operand, n on partitions)
  - qT [ct, p, j, c] = q[ct*128+c, j*128+p]  (gemm1 stationary, per c-tile)
  - kn [p, dt, n] = kv[dt*128+p, n] (gemm2 moving operand, natural layout)
Each layout is partition-major so every load DMA moves 8KB-contiguous
descriptor payloads (full bus efficiency, no 2x small-element penalty).

Device program per batch (software-pipelined over c-tiles):
  g1(ct):  energy = qT(ct)^T @ ktT  (32 fp16 matmuls into one PSUM bank)
           min-reduce (DVE), att16 = exp(min - E) with sum accum (ACT),
           normalize att16 (DVE), attT via small DMA xbar transpose.
  g2(ct):  out = attT^T @ kn  (8 x 4 fp16 matmuls), exp (ACT, fp16 out),
           per-W-segment sums (DVE), reciprocal (DVE), normalize
           (alternating DVE/Pool), fp16 store per half-c-tile.
PE stream order is g1(0), g1(1), g2(0), g1(2), g2(1), ... so the in-order PE
never stalls waiting for the attT DMA round trip. Next batch's loads are
emitted between the current batch's tasks so the (serialized) DMA engines
prefetch ktT/kn/qT behind the compute.

gemm1 runs in plain fp16 (host-rounded inputs, fp32 PSUM accumulation):
measured absmax rel err ~1.2e-2 against the f64 reference (gate 2e-2).
G1_MODE="f16q" adds a q-residual pass (err ~7e-3) at +27us PE if needed.
"""

import sys

import numpy as np

for _p in ("/opt/trn_rl_repo", "/root/.axon_site/_ro/trn_rl_repo"):
    if _p not in sys.path:
        sys.path.append(_p)

B = 16
N_CORES = 8
B_PER_CORE = B // N_CORES
C = 512
H = 64
W = 64
N = H * W
CT = C // 128  # 4 c-tiles / d-tiles
NJ = N // 128  # 32 n-chunks of 128

G1_MODE = "f16"  # "f16" (plain fp16) | "f16q" (q hi/lo split, 2-pass)


def build_program(g1_mode=None):
    from contextlib import ExitStack

    import concourse.mybir as mybir
    import concourse.tile as tile
    from concourse import bacc

    if g1_mode is None:
        g1_mode = G1_MODE
    assert g1_mode in ("f16", "f16q")
    q_split = g1_mode == "f16q"

    f32 = mybir.dt.float32
    f16 = mybir.dt.float16
    Alu = mybir.AluOpType
    Act = mybir.ActivationFunctionType
    Axis = mybir.AxisListType

    nc = bacc.Bacc()
    # Host-prepared layouts (see module docstring).
    ktT = nc.declare_dram_parameter("ktT", [B_PER_CORE, 128, NJ, C], f16, isOutput=False)
    qT = nc.declare_dram_parameter("qT", [B_PER_CORE, CT, 128, NJ, 128], f16, isOutput=False)
    if q_split:
        qlT = nc.declare_dram_parameter(
            "qlT", [B_PER_CORE, CT, 128, NJ, 128], f16, isOutput=False
        )
    kn = nc.declare_dram_parameter("kn", [B_PER_CORE, 128, CT, N], f16, isOutput=False)
    out = nc.declare_dram_parameter("out", [B_PER_CORE, C, N], f16, isOutput=True)

    with tile.TileContext(nc) as tc, ExitStack() as ctx:
        ktp = ctx.enter_context(tc.tile_pool(name="ktp", bufs=2))
        knp = ctx.enter_context(tc.tile_pool(name="knp", bufs=2))
        qtp = ctx.enter_context(tc.tile_pool(name="qtp", bufs=3 + 3 * q_split))
        attp = ctx.enter_context(tc.tile_pool(name="attp", bufs=2))
        ostp = ctx.enter_context(tc.tile_pool(name="ostp", bufs=3))
        small = ctx.enter_context(tc.tile_pool(name="small", bufs=4))
        ps_e = ctx.enter_context(tc.tile_pool(name="ps_e", bufs=2, space="PSUM"))
        ps_o = ctx.enter_context(tc.tile_pool(name="ps_o", bufs=4, space="PSUM"))

        # Per-batch SBUF tiles, created/rotated on demand.
        kt_sb = {}
        kn_sb = {}
        qt_sb = {}
        ql_sb = {}
        att_T = {}

        def emit_ktT_load(b):
            kt_sb[b] = ktp.tile([128, NJ, C], f16, tag="ktT", name=f"ktT_{b}")
            for g in range(4):
                js = slice(g * (NJ // 4), (g + 1) * (NJ // 4))
                nc.sync.dma_start(out=kt_sb[b][:, js, :], in_=ktT[b, :, js, :])

        def emit_kn_load(b):
            kn_sb[b] = knp.tile([128, CT, N], f16, tag="kn", name=f"kn_{b}")
            for dt in range(CT):
                nc.sync.dma_start(out=kn_sb[b][:, dt, :], in_=kn[b, :, dt, :])

        def emit_qT_load(b, ct):
            qt_sb[(b, ct)] = qtp.tile([128, NJ, 128], f16, tag="qT", name=f"qT_{b}_{ct}")
            nc.sync.dma_start(out=qt_sb[(b, ct)], in_=qT[b, ct])
            if q_split:
                ql_sb[(b, ct)] = qtp.tile(
                    [128, NJ, 128], f16, tag="qlT", name=f"qlT_{b}_{ct}"
                )
                nc.sync.dma_start(out=ql_sb[(b, ct)], in_=qlT[b, ct])

        def emit_g1(b, ct):
            # energy for this c-tile, then attention row softmax + transpose
            e_ps = ps_e.tile([128, C], f32, tag="ps_e")
            qt = qt_sb.pop((b, ct))
            ql = ql_sb.pop((b, ct), None)
            kt = kt_sb[b]
            for j in range(NJ):
                last = j == NJ - 1
                nc.tensor.matmul(
                    e_ps, qt[:, j, :], kt[:, j, :],
                    start=(j == 0), stop=(last and not q_split),
                )
                if q_split:
                    nc.tensor.matmul(
                        e_ps, ql[:, j, :], kt[:, j, :], start=False, stop=last
                    )
            min_t = small.tile([128, 1], f32, tag="min")
            nc.vector.tensor_reduce(min_t, e_ps, axis=Axis.X, op=Alu.min)
            att16 = attp.tile([128, C], f16, tag="att16")
            den = small.tile([128, 1], f32, tag="den")
            nc.scalar.activation(
                out=att16, in_=e_ps, func=Act.Exp, bias=min_t, scale=-1.0,
                accum_out=den,
            )
            rden = small.tile([128, 1], f32, tag="rden")
            nc.vector.reciprocal(rden, den)
            nc.vector.tensor_scalar_mul(att16, att16, rden)
            att_T[(b, ct)] = attp.tile(
                [128, CT, 128], f16, tag="attT", name=f"attT_{b}_{ct}"
            )
            nc.sync.dma_start_transpose(att_T[(b, ct)], att16)

        def emit_g2(b, ct):
            # out rows for this c-tile: gemm2 + final softmax over W + store
            attT = att_T.pop((b, ct))
            knb = kn_sb[b]
            for h in range(2):
                o16 = ostp.tile([128, 2048], f16, tag="ost")  # 4 nj chunks of 512
                for k in range(4):
                    nj = h * 4 + k
                    o_ps = ps_o.tile([128, 512], f32, tag="ps_o")
                    for dt in range(CT):
                        nc.tensor.matmul(
                            o_ps,
                            attT[:, dt, :],
                            knb[:, dt, nj * 512 : (nj + 1) * 512],
                            start=(dt == 0),
                            stop=(dt == CT - 1),
                        )
                    o16v = o16[:, k * 512 : (k + 1) * 512].rearrange(
                        "p (s w) -> p s w", w=W
                    )
                    nc.scalar.activation(
                        out=o16v,
                        in_=o_ps.rearrange("p (s w) -> p s w", w=W),
                        func=Act.Exp,
                    )
                    ssum = small.tile([128, 512 // W], f32, tag="ssum")
                    nc.vector.tensor_reduce(ssum, o16v, axis=Axis.X, op=Alu.add)
                    rsum = small.tile([128, 512 // W], f32, tag="rsum")
                    nc.vector.reciprocal(rsum, ssum)
                    eng = nc.vector if nj % 2 == 0 else nc.gpsimd
                    eng.tensor_tensor(
                        out=o16v,
                        in0=o16v,
                        in1=rsum[:, :, None].to_broadcast(o16v.shape),
                        op=Alu.mult,
                    )
                nc.sync.dma_start(
                    out=out[b, ct * 128 : (ct + 1) * 128, h * 2048 : (h + 1) * 2048],
                    in_=o16,
                )

        # ---- software-pipelined schedule ----
        tasks = [(b, ct) for b in range(B_PER_CORE) for ct in range(CT)]
        emit_ktT_load(0)
        emit_qT_load(0, 0)
        emit_qT_load(0, 1)
        emit_kn_load(0)
        emit_qT_load(0, 2)
        emit_qT_load(0, 3)
        for i, (b, ct) in enumerate(tasks):
            emit_g1(b, ct)
            # prefetch next batch's tensors between this batch's tasks
            if b + 1 < B_PER_CORE:
                if ct == 0:
                    emit_ktT_load(b + 1)
                elif ct == 1:
                    emit_kn_load(b + 1)
                elif ct == 2:
                    emit_qT_load(b + 1, 0)
                    emit_qT_load(b + 1, 1)
                elif ct == 3:
                    emit_qT_load(b + 1, 2)
                    emit_qT_load(b + 1, 3)
            if i > 0:
                emit_g2(*tasks[i - 1])
        emit_g2(*tasks[-1])

    nc.finalize()
    return nc


def prepare_in_maps(x_training, x_pre, g1_mode=None):
    """Host-side shard + layout prep. Returns per-core input dicts."""
    if g1_mode is None:
        g1_mode = G1_MODE
    xt = np.asarray(x_training, dtype=np.float32).reshape(B, C, N)
    xp = np.asarray(x_pre, dtype=np.float32).reshape(B, C, N)
    xt16 = xt.astype(np.float16)
    xp16 = xp.astype(np.float16)

    # ktT[b, p, j, d] = kv[b, d, j*128+p]
    ktT = np.ascontiguousarray(
        xt16.reshape(B, C, NJ, 128).transpose(0, 3, 2, 1)
    )
    # qT[b, ct, p, j, c] = q[b, ct*128+c, j*128+p]
    qT = np.ascontiguousarray(
        xp16.reshape(B, CT, 128, NJ, 128).transpose(0, 1, 4, 3, 2)
    )
    # kn[b, p, dt, n] = kv[b, dt*128+p, n]
    knat = np.ascontiguousarray(xt16.reshape(B, CT, 128, N).transpose(0, 2, 1, 3))
    if g1_mode == "f16q":
        ql = (xp - xp16.astype(np.float32)).astype(np.float16)
        qlT = np.ascontiguousarray(
            ql.reshape(B, CT, 128, NJ, 128).transpose(0, 1, 4, 3, 2)
        )

    in_maps = []
    for i in range(N_CORES):
        sl = slice(i * B_PER_CORE, (i + 1) * B_PER_CORE)
        m = {"ktT": ktT[sl], "qT": qT[sl], "kn": knat[sl]}
        if g1_mode == "f16q":
            m["qlT"] = qlT[sl]
        in_maps.append(m)
    return in_maps


def kernel(x_training: np.ndarray, x_pre: np.ndarray) -> np.ndarray:
    from concourse.bass_utils import run_bass_kernel_spmd

    nc = build_program()
    in_maps = prepare_in_maps(x_training, x_pre)
    res = run_bass_kernel_spmd(nc, in_maps, list(range(N_CORES)))
    outs = [np.asarray(r["out"]) for r in res.results]
    return np.concatenate(outs, axis=0).reshape(B, C, H, W).astype(np.float32)


# revision 9
# speedup vs baseline: 1.0689x; 1.0689x over previous
"""ChannelGuidedAttn Trainium2 kernel.

Reference computation (per batch b):
    q  = x_pre[b]      reshaped (C, N),  C=512, N=H*W=4096
    kv = x_training[b] reshaped (C, N)
    energy[c,d] = <q[c,:], kv[d,:]>                      (C x C)
    att = softmax(max_d(energy) - energy, axis=-1)       == softmax(-energy)
        = exp(min_d(energy) - energy) / sum
    out = att @ kv  -> (C, H, W);  final softmax over W

Sharding: data-parallel over batch B=16 across 8 cores (2 batches/core).

Design (v2): all transposes/casts are done on the HOST during sharding; the
device receives three pre-laid-out fp16 tensors per batch:
  - ktT[p, j, d] = kv[d, j*128+p]   (gemm1 moving operand, n on partitions)
  - qT [ct, p, j, c] = q[ct*128+c, j*128+p]  (gemm1 stationary, per c-tile)
  - kn [p, dt, n] = kv[dt*128+p, n] (gemm2 moving operand, natural layout)
Each layout is partition-major so every load DMA moves 8KB-contiguous
descriptor payloads (full bus efficiency, no 2x small-element penalty).

Device program per batch (software-pipelined over c-tiles):
  g1(ct):  energy = qT(ct)^T @ ktT  (32 fp16 matmuls into one PSUM bank)
           min-reduce (DVE), att16 = exp(min - E) with sum accum (ACT),
           normalize att16 (DVE), attT via small DMA xbar transpose.
  g2(ct):  out = attT^T @ kn  (8 x 4 fp16 matmuls), exp (ACT, fp16 out),
           per-W-segment sums (DVE), reciprocal (DVE), normalize
           (alternating DVE/Pool), fp16 store per half-c-tile.
PE stream order is g1(0), g1(1), g2(0), g1(2), g2(1), ... so the in-order PE
never stalls waiting for the attT DMA round trip. Next batch's loads are
emitted between the current batch's tasks so the (serialized) DMA engines
prefetch ktT/kn/qT behind the compute.

gemm1 runs in plain fp16 (host-rounded inputs, fp32 PSUM accumulation):
measured absmax rel err ~1.2e-2 against the f64 reference (gate 2e-2).
G1_MODE="f16q" adds a q-residual pass (err ~7e-3) at +27us PE if needed.
"""

import sys

import numpy as np

for _p in ("/opt/trn_rl_repo", "/root/.axon_site/_ro/trn_rl_repo"):
    if _p not in sys.path:
        sys.path.append(_p)

B = 16
N_CORES = 8
B_PER_CORE = B // N_CORES
C = 512
H = 64
W = 64
N = H * W
CT = C // 128  # 4 c-tiles / d-tiles
NJ = N // 128  # 32 n-chunks of 128

G1_MODE = "f16"  # "f16" (plain fp16) | "f16q" (q hi/lo split, 2-pass)


def build_program(g1_mode=None):
    from contextlib import ExitStack

    import concourse.mybir as mybir
    import concourse.tile as tile
    from concourse import bacc

    if g1_mode is None:
        g1_mode = G1_MODE
    assert g1_mode in ("f16", "f16q")
    q_split = g1_mode == "f16q"

    f32 = mybir.dt.float32
    f16 = mybir.dt.float16
    Alu = mybir.AluOpType
    Act = mybir.ActivationFunctionType
    Axis = mybir.AxisListType

    nc = bacc.Bacc()
    # Host-prepared layouts (see module docstring).
    ktT = nc.declare_dram_parameter("ktT", [B_PER_CORE, 128, NJ, C], f16, isOutput=False)
    qT = nc.declare_dram_parameter("qT", [B_PER_CORE, CT, 128, NJ, 128], f16, isOutput=False)
    if q_split:
        qlT = nc.declare_dram_parameter(
            "qlT", [B_PER_CORE, CT, 128, NJ, 128], f16, isOutput=False
        )
    kn = nc.declare_dram_parameter("kn", [B_PER_CORE, 128, CT, N], f16, isOutput=False)
    out = nc.declare_dram_parameter("out", [B_PER_CORE, C, N], f16, isOutput=True)

    with tile.TileContext(nc) as tc, ExitStack() as ctx:
        ktp = ctx.enter_context(tc.tile_pool(name="ktp", bufs=2))
        knp = ctx.enter_context(tc.tile_pool(name="knp", bufs=2))
        qtp = ctx.enter_context(tc.tile_pool(name="qtp", bufs=5 + 3 * q_split))
        attp = ctx.enter_context(tc.tile_pool(name="attp", bufs=2))
        ostp = ctx.enter_context(tc.tile_pool(name="ostp", bufs=3))
        small = ctx.enter_context(tc.tile_pool(name="small", bufs=4))
        ps_e = ctx.enter_context(tc.tile_pool(name="ps_e", bufs=3, space="PSUM"))
        ps_o = ctx.enter_context(tc.tile_pool(name="ps_o", bufs=4, space="PSUM"))

        # Per-batch SBUF tiles, created/rotated on demand.
        kt_sb = {}
        kn_sb = {}
        qt_sb = {}
        ql_sb = {}
        att_T = {}

        def emit_ktT_chunks(b, chunks):
            # 8 chunks of 4 j's each (~1.6us apiece on the DMA queue)
            if b not in kt_sb:
                kt_sb[b] = ktp.tile([128, NJ, C], f16, tag="ktT", name=f"ktT_{b}")
            for g in chunks:
                js = slice(g * (NJ // 8), (g + 1) * (NJ // 8))
                nc.sync.dma_start(out=kt_sb[b][:, js, :], in_=ktT[b, :, js, :])

        def emit_kn_chunks(b, chunks):
            if b not in kn_sb:
                kn_sb[b] = knp.tile([128, CT, N], f16, tag="kn", name=f"kn_{b}")
            for dt in chunks:
                nc.sync.dma_start(out=kn_sb[b][:, dt, :], in_=kn[b, :, dt, :])

        def emit_qT_load(b, ct):
            qt_sb[(b, ct)] = qtp.tile([128, NJ, 128], f16, tag="qT", name=f"qT_{b}_{ct}")
            nc.sync.dma_start(out=qt_sb[(b, ct)], in_=qT[b, ct])
            if q_split:
                ql_sb[(b, ct)] = qtp.tile(
                    [128, NJ, 128], f16, tag="qlT", name=f"qlT_{b}_{ct}"
                )
                nc.sync.dma_start(out=ql_sb[(b, ct)], in_=qlT[b, ct])

        def emit_g1(b, ct):
            # energy for this c-tile, then attention row softmax + transpose
            e_ps = ps_e.tile([128, C], f32, tag="ps_e")
            qt = qt_sb.pop((b, ct))
            ql = ql_sb.pop((b, ct), None)
            kt = kt_sb[b]
            for j in range(NJ):
                last = j == NJ - 1
                nc.tensor.matmul(
                    e_ps, qt[:, j, :], kt[:, j, :],
                    start=(j == 0), stop=(last and not q_split),
                )
                if q_split:
                    nc.tensor.matmul(
                        e_ps, ql[:, j, :], kt[:, j, :], start=False, stop=last
                    )
            min_t = small.tile([128, 1], f32, tag="min")
            nc.vector.tensor_reduce(min_t, e_ps, axis=Axis.X, op=Alu.min)
            att16 = attp.tile([128, C], f16, tag="att16")
            den = small.tile([128, 1], f32, tag="den")
            nc.scalar.activation(
                out=att16, in_=e_ps, func=Act.Exp, bias=min_t, scale=-1.0,
                accum_out=den,
            )
            rden = small.tile([128, 1], f32, tag="rden")
            nc.vector.reciprocal(rden, den)
            nc.vector.tensor_scalar_mul(att16, att16, rden)
            att_T[(b, ct)] = attp.tile(
                [128, CT, 128], f16, tag="attT", name=f"attT_{b}_{ct}"
            )
            nc.sync.dma_start_transpose(att_T[(b, ct)], att16)

        def emit_g2(b, ct, last=False):
            # out rows for this c-tile: gemm2 + final softmax over W + store
            attT = att_T.pop((b, ct))
            knb = kn_sb[b]
            for h in range(2):
                o16 = ostp.tile([128, 2048], f16, tag="ost")  # 4 nj chunks of 512
                for k in range(4):
                    nj = h * 4 + k
                    o_ps = ps_o.tile([128, 512], f32, tag="ps_o")
                    for dt in range(CT):
                        nc.tensor.matmul(
                            o_ps,
                            attT[:, dt, :],
                            knb[:, dt, nj * 512 : (nj + 1) * 512],
                            start=(dt == 0),
                            stop=(dt == CT - 1),
                        )
                    o16v = o16[:, k * 512 : (k + 1) * 512].rearrange(
                        "p (s w) -> p s w", w=W
                    )
                    nc.scalar.activation(
                        out=o16v,
                        in_=o_ps.rearrange("p (s w) -> p s w", w=W),
                        func=Act.Exp,
                    )
                    ssum = small.tile([128, 512 // W], f32, tag="ssum")
                    nc.vector.tensor_reduce(ssum, o16v, axis=Axis.X, op=Alu.add)
                    rsum = small.tile([128, 512 // W], f32, tag="rsum")
                    nc.vector.reciprocal(rsum, ssum)
                    # keep the last, latency-critical chunks on DVE (faster
                    # per-op than Pool); Pool takes the even chunks
                    eng = nc.gpsimd if nj % 2 == 0 and not last else nc.vector
                    eng.tensor_tensor(
                        out=o16v,
                        in0=o16v,
                        in1=rsum[:, :, None].to_broadcast(o16v.shape),
                        op=Alu.mult,
                    )
                    if last:
                        # per-nj stores to shorten the kernel tail
                        nc.sync.dma_start(
                            out=out[
                                b,
                                ct * 128 : (ct + 1) * 128,
                                nj * 512 : (nj + 1) * 512,
                            ],
                            in_=o16[:, k * 512 : (k + 1) * 512],
                        )
                if not last:
                    nc.sync.dma_start(
                        out=out[
                            b, ct * 128 : (ct + 1) * 128, h * 2048 : (h + 1) * 2048
                        ],
                        in_=o16,
                    )

        # ---- explicit software-pipelined schedule (2 batches) ----
        # DMA queue and the in-order PE stream are co-scheduled: batch 0 runs
        # all four gemm1s first (its kn/attT can't be ready earlier anyway);
        # batch 1 prefetches are slotted so no PE task ever waits on a load.
        assert B_PER_CORE == 2
        emit_qT_load(0, 0)
        emit_ktT_chunks(0, range(0, 4))
        emit_qT_load(0, 1)
        emit_ktT_chunks(0, range(4, 8))
        emit_qT_load(0, 2)
        emit_qT_load(0, 3)
        emit_kn_chunks(0, range(CT))

        emit_g1(0, 0)
        emit_qT_load(1, 0)
        emit_ktT_chunks(1, range(0, 2))
        emit_g1(0, 1)
        emit_ktT_chunks(1, range(2, 4))
        emit_g1(0, 2)
        emit_ktT_chunks(1, range(4, 6))
        emit_g1(0, 3)
        emit_ktT_chunks(1, range(6, 8))
        emit_kn_chunks(1, [0, 1])
        emit_g2(0, 0)
        emit_kn_chunks(1, [2])
        emit_qT_load(1, 1)
        emit_g2(0, 1)
        emit_kn_chunks(1, [3])
        emit_g2(0, 2)
        emit_qT_load(1, 2)
        emit_qT_load(1, 3)
        emit_g2(0, 3)
        emit_g1(1, 0)
        emit_g1(1, 1)
        emit_g2(1, 0)
        emit_g1(1, 2)
        emit_g2(1, 1)
        emit_g1(1, 3)
        emit_g2(1, 2)
        emit_g2(1, 3, last=True)

    nc.finalize()
    return nc


def prepare_in_maps(x_training, x_pre, g1_mode=None):
    """Host-side shard + layout prep. Returns per-core input dicts."""
    if g1_mode is None:
        g1_mode = G1_MODE
    xt = np.asarray(x_training, dtype=np.float32).reshape(B, C, N)
    xp = np.asarray(x_pre, dtype=np.float32).reshape(B, C, N)
    xt16 = xt.astype(np.float16)
    xp16 = xp.astype(np.float16)

    # ktT[b, p, j, d] = kv[b, d, j*128+p]
    ktT = np.ascontiguousarray(
        xt16.reshape(B, C, NJ, 128).transpose(0, 3, 2, 1)
    )
    # qT[b, ct, p, j, c] = q[b, ct*128+c, j*128+p]
    qT = np.ascontiguousarray(
        xp16.reshape(B, CT, 128, NJ, 128).transpose(0, 1, 4, 3, 2)
    )
    # kn[b, p, dt, n] = kv[b, dt*128+p, n]
    knat = np.ascontiguousarray(xt16.reshape(B, CT, 128, N).transpose(0, 2, 1, 3))
    if g1_mode == "f16q":
        ql = (xp - xp16.astype(np.float32)).astype(np.float16)
        qlT = np.ascontiguousarray(
            ql.reshape(B, CT, 128, NJ, 128).transpose(0, 1, 4, 3, 2)
        )

    in_maps = []
    for i in range(N_CORES):
        sl = slice(i * B_PER_CORE, (i + 1) * B_PER_CORE)
        m = {"ktT": ktT[sl], "qT": qT[sl], "kn": knat[sl]}
        if g1_mode == "f16q":
            m["qlT"] = qlT[sl]
        in_maps.append(m)
    return in_maps


def kernel(x_training: np.ndarray, x_pre: np.ndarray) -> np.ndarray:
    from concourse.bass_utils import run_bass_kernel_spmd

    nc = build_program()
    in_maps = prepare_in_maps(x_training, x_pre)
    res = run_bass_kernel_spmd(nc, in_maps, list(range(N_CORES)))
    outs = [np.asarray(r["out"]) for r in res.results]
    return np.concatenate(outs, axis=0).reshape(B, C, H, W).astype(np.float32)


# revision 13
# speedup vs baseline: 1.1059x; 1.0346x over previous
"""ChannelGuidedAttn Trainium2 kernel.

Reference computation (per batch b):
    q  = x_pre[b]      reshaped (C, N),  C=512, N=H*W=4096
    kv = x_training[b] reshaped (C, N)
    energy[c,d] = <q[c,:], kv[d,:]>                      (C x C)
    att = softmax(max_d(energy) - energy, axis=-1)       == softmax(-energy)
        = exp(min_d(energy) - energy) / sum
    out = att @ kv  -> (C, H, W);  final softmax over W

Sharding: data-parallel over batch B=16 across 8 cores (2 batches/core).

Design (v2): all transposes/casts are done on the HOST during sharding; the
device receives three pre-laid-out fp16 tensors per batch:
  - ktT[p, j, d] = kv[d, j*128+p]   (gemm1 moving operand, n on partitions)
  - qT [ct, p, j, c] = q[ct*128+c, j*128+p]  (gemm1 stationary, per c-tile)
  - kn [p, dt, n] = kv[dt*128+p, n] (gemm2 moving operand, natural layout)
Each layout is partition-major so every load DMA moves 8KB-contiguous
descriptor payloads (full bus efficiency, no 2x small-element penalty).

Device program per batch (software-pipelined over c-tiles):
  g1(ct):  energy = qT(ct)^T @ ktT  (32 fp16 matmuls into one PSUM bank)
           min-reduce (DVE), att16 = exp(min - E) with sum accum (ACT),
           normalize att16 (DVE), attT via small DMA xbar transpose.
  g2(ct):  out = attT^T @ kn  (8 x 4 fp16 matmuls), exp (ACT, fp16 out),
           per-W-segment sums (DVE), reciprocal (DVE), normalize
           (alternating DVE/Pool), fp16 store per half-c-tile.
PE stream order is g1(0), g1(1), g2(0), g1(2), g2(1), ... so the in-order PE
never stalls waiting for the attT DMA round trip. Next batch's loads are
emitted between the current batch's tasks so the (serialized) DMA engines
prefetch ktT/kn/qT behind the compute.

gemm1 runs in plain fp16 (host-rounded inputs, fp32 PSUM accumulation):
measured absmax rel err ~1.2e-2 against the f64 reference (gate 2e-2).
G1_MODE="f16q" adds a q-residual pass (err ~7e-3) at +27us PE if needed.
"""

import sys

import numpy as np

for _p in ("/opt/trn_rl_repo", "/root/.axon_site/_ro/trn_rl_repo"):
    if _p not in sys.path:
        sys.path.append(_p)

B = 16
N_CORES = 8
B_PER_CORE = B // N_CORES
C = 512
H = 64
W = 64
N = H * W
CT = C // 128  # 4 c-tiles / d-tiles
NJ = N // 128  # 32 n-chunks of 128

G1_MODE = "f16"  # "f16" (plain fp16) | "f16q" (q hi/lo split, 2-pass)


def build_program(g1_mode=None):
    from contextlib import ExitStack

    import concourse.mybir as mybir
    import concourse.tile as tile
    from concourse import bacc

    if g1_mode is None:
        g1_mode = G1_MODE
    assert g1_mode in ("f16", "f16q")
    q_split = g1_mode == "f16q"

    f32 = mybir.dt.float32
    f16 = mybir.dt.float16
    Alu = mybir.AluOpType
    Act = mybir.ActivationFunctionType
    Axis = mybir.AxisListType

    nc = bacc.Bacc()
    # Host-prepared layouts (see module docstring).
    ktT = nc.declare_dram_parameter("ktT", [B_PER_CORE, 128, NJ, C], f16, isOutput=False)
    qT = nc.declare_dram_parameter("qT", [B_PER_CORE, CT, 128, NJ, 128], f16, isOutput=False)
    if q_split:
        qlT = nc.declare_dram_parameter(
            "qlT", [B_PER_CORE, CT, 128, NJ, 128], f16, isOutput=False
        )
    kn = nc.declare_dram_parameter("kn", [B_PER_CORE, 128, CT, N], f16, isOutput=False)
    out = nc.declare_dram_parameter("out", [B_PER_CORE, C, N], f16, isOutput=True)

    with tile.TileContext(nc) as tc, ExitStack() as ctx:
        ktp = ctx.enter_context(tc.tile_pool(name="ktp", bufs=2))
        knp = ctx.enter_context(tc.tile_pool(name="knp", bufs=2))
        qtp = ctx.enter_context(tc.tile_pool(name="qtp", bufs=5 + 3 * q_split))
        attp = ctx.enter_context(tc.tile_pool(name="attp", bufs=2))
        ostp = ctx.enter_context(tc.tile_pool(name="ostp", bufs=5))
        small = ctx.enter_context(tc.tile_pool(name="small", bufs=4))
        ps_e = ctx.enter_context(tc.tile_pool(name="ps_e", bufs=3, space="PSUM"))
        ps_o = ctx.enter_context(tc.tile_pool(name="ps_o", bufs=4, space="PSUM"))

        # Per-batch SBUF tiles, created/rotated on demand.
        kt_sb = {}
        kn_sb = {}
        qt_sb = {}
        ql_sb = {}
        att_T = {}

        def emit_ktT_chunks(b, chunks):
            # 8 chunks of 4 j's each (~1.6us apiece on the DMA queue)
            if b not in kt_sb:
                kt_sb[b] = ktp.tile([128, NJ, C], f16, tag="ktT", name=f"ktT_{b}")
            for g in chunks:
                js = slice(g * (NJ // 8), (g + 1) * (NJ // 8))
                nc.sync.dma_start(out=kt_sb[b][:, js, :], in_=ktT[b, :, js, :])

        def emit_kn_chunks(b, chunks):
            if b not in kn_sb:
                kn_sb[b] = knp.tile([128, CT, N], f16, tag="kn", name=f"kn_{b}")
            for dt in chunks:
                nc.sync.dma_start(out=kn_sb[b][:, dt, :], in_=kn[b, :, dt, :])

        def emit_qT_load(b, ct, halves=1):
            qt_sb[(b, ct)] = qtp.tile([128, NJ, 128], f16, tag="qT", name=f"qT_{b}_{ct}")
            for hh in range(halves):
                js = slice(hh * (NJ // halves), (hh + 1) * (NJ // halves))
                nc.sync.dma_start(out=qt_sb[(b, ct)][:, js, :], in_=qT[b, ct, :, js, :])
            if q_split:
                ql_sb[(b, ct)] = qtp.tile(
                    [128, NJ, 128], f16, tag="qlT", name=f"qlT_{b}_{ct}"
                )
                nc.sync.dma_start(out=ql_sb[(b, ct)], in_=qlT[b, ct])

        def emit_g1(b, ct):
            # energy for this c-tile, then attention row softmax + transpose
            e_ps = ps_e.tile([128, C], f32, tag="ps_e")
            qt = qt_sb.pop((b, ct))
            ql = ql_sb.pop((b, ct), None)
            kt = kt_sb[b]
            for j in range(NJ):
                last = j == NJ - 1
                nc.tensor.matmul(
                    e_ps, qt[:, j, :], kt[:, j, :],
                    start=(j == 0), stop=(last and not q_split),
                )
                if q_split:
                    nc.tensor.matmul(
                        e_ps, ql[:, j, :], kt[:, j, :], start=False, stop=last
                    )
            min_t = small.tile([128, 1], f32, tag="min")
            nc.vector.tensor_reduce(min_t, e_ps, axis=Axis.X, op=Alu.min)
            att16 = attp.tile([128, C], f16, tag="att16")
            den = small.tile([128, 1], f32, tag="den")
            nc.scalar.activation(
                out=att16, in_=e_ps, func=Act.Exp, bias=min_t, scale=-1.0,
                accum_out=den,
            )
            rden = small.tile([128, 1], f32, tag="rden")
            nc.vector.reciprocal(rden, den)
            nc.vector.tensor_scalar_mul(att16, att16, rden)
            att_T[(b, ct)] = attp.tile(
                [128, CT, 128], f16, tag="attT", name=f"attT_{b}_{ct}"
            )
            nc.sync.dma_start_transpose(att_T[(b, ct)], att16)

        def emit_g2(b, ct, tail=False, last=False):
            # out rows for this c-tile: gemm2 + final softmax over W + store.
            # tail: the final two tasks — DVE is the scarce engine there, so
            # push most normalizes to Pool (DVE keeps nj 5,7 for low latency).
            attT = att_T.pop((b, ct))
            knb = kn_sb[b]
            for h in range(2):
                o16 = ostp.tile([128, 2048], f16, tag="ost")  # 4 nj chunks of 512
                for k in range(4):
                    nj = h * 4 + k
                    o_ps = ps_o.tile([128, 512], f32, tag="ps_o")
                    for dt in range(CT):
                        nc.tensor.matmul(
                            o_ps,
                            attT[:, dt, :],
                            knb[:, dt, nj * 512 : (nj + 1) * 512],
                            start=(dt == 0),
                            stop=(dt == CT - 1),
                        )
                    o16v = o16[:, k * 512 : (k + 1) * 512].rearrange(
                        "p (s w) -> p s w", w=W
                    )
                    nc.scalar.activation(
                        out=o16v,
                        in_=o_ps.rearrange("p (s w) -> p s w", w=W),
                        func=Act.Exp,
                    )
                    ssum = small.tile([128, 512 // W], f32, tag="ssum")
                    nc.vector.tensor_reduce(ssum, o16v, axis=Axis.X, op=Alu.add)
                    rsum = small.tile([128, 512 // W], f32, tag="rsum")
                    nc.vector.reciprocal(rsum, ssum)
                    if tail:
                        on_pool = nj not in (5, 7)
                    else:
                        on_pool = nj % 2 == 0
                    eng = nc.gpsimd if on_pool else nc.vector
                    eng.tensor_tensor(
                        out=o16v,
                        in0=o16v,
                        in1=rsum[:, :, None].to_broadcast(o16v.shape),
                        op=Alu.mult,
                    )
                    if last:
                        # per-nj stores to shorten the kernel tail
                        nc.sync.dma_start(
                            out=out[
                                b,
                                ct * 128 : (ct + 1) * 128,
                                nj * 512 : (nj + 1) * 512,
                            ],
                            in_=o16[:, k * 512 : (k + 1) * 512],
                        )
                if not last:
                    nc.sync.dma_start(
                        out=out[
                            b, ct * 128 : (ct + 1) * 128, h * 2048 : (h + 1) * 2048
                        ],
                        in_=o16,
                    )

        # ---- explicit software-pipelined schedule (2 batches) ----
        # DMA queue and the in-order PE stream are co-scheduled: batch 0 runs
        # all four gemm1s first (its kn/attT can't be ready earlier anyway);
        # batch 1 prefetches are slotted so no PE task ever waits on a load.
        assert B_PER_CORE == 2
        emit_qT_load(0, 0, halves=2)
        emit_ktT_chunks(0, range(0, 4))
        emit_qT_load(0, 1)
        emit_ktT_chunks(0, range(4, 8))
        emit_qT_load(0, 2)
        emit_qT_load(0, 3)
        emit_kn_chunks(0, range(CT))

        emit_g1(0, 0)
        emit_qT_load(1, 0)
        emit_ktT_chunks(1, range(0, 2))
        emit_g1(0, 1)
        emit_ktT_chunks(1, range(2, 4))
        emit_g1(0, 2)
        emit_ktT_chunks(1, range(4, 6))
        emit_g1(0, 3)
        emit_ktT_chunks(1, range(6, 8))
        emit_g2(0, 0)  # stores go ahead of the kn1 prefetch in the queue
        emit_kn_chunks(1, [0, 1])
        emit_qT_load(1, 1)
        emit_g2(0, 1)
        emit_kn_chunks(1, [2, 3])
        emit_g2(0, 2)
        emit_qT_load(1, 2)
        emit_qT_load(1, 3)
        emit_g2(0, 3)
        emit_g1(1, 0)
        emit_g1(1, 1)
        emit_g2(1, 0)
        emit_g1(1, 2)
        emit_g2(1, 1)
        emit_g1(1, 3)
        emit_g2(1, 2, tail=True)
        emit_g2(1, 3, tail=True, last=True)

    nc.finalize()
    return nc


def prepare_in_maps(x_training, x_pre, g1_mode=None):
    """Host-side shard + layout prep. Returns per-core input dicts."""
    if g1_mode is None:
        g1_mode = G1_MODE
    xt = np.asarray(x_training, dtype=np.float32).reshape(B, C, N)
    xp = np.asarray(x_pre, dtype=np.float32).reshape(B, C, N)
    xt16 = xt.astype(np.float16)
    xp16 = xp.astype(np.float16)

    # ktT[b, p, j, d] = kv[b, d, j*128+p]
    ktT = np.ascontiguousarray(
        xt16.reshape(B, C, NJ, 128).transpose(0, 3, 2, 1)
    )
    # qT[b, ct, p, j, c] = q[b, ct*128+c, j*128+p]
    qT = np.ascontiguousarray(
        xp16.reshape(B, CT, 128, NJ, 128).transpose(0, 1, 4, 3, 2)
    )
    # kn[b, p, dt, n] = kv[b, dt*128+p, n]
    knat = np.ascontiguousarray(xt16.reshape(B, CT, 128, N).transpose(0, 2, 1, 3))
    if g1_mode == "f16q":
        ql = (xp - xp16.astype(np.float32)).astype(np.float16)
        qlT = np.ascontiguousarray(
            ql.reshape(B, CT, 128, NJ, 128).transpose(0, 1, 4, 3, 2)
        )

    in_maps = []
    for i in range(N_CORES):
        sl = slice(i * B_PER_CORE, (i + 1) * B_PER_CORE)
        m = {"ktT": ktT[sl], "qT": qT[sl], "kn": knat[sl]}
        if g1_mode == "f16q":
            m["qlT"] = qlT[sl]
        in_maps.append(m)
    return in_maps


def kernel(x_training: np.ndarray, x_pre: np.ndarray) -> np.ndarray:
    from concourse.bass_utils import run_bass_kernel_spmd

    nc = build_program()
    in_maps = prepare_in_maps(x_training, x_pre)
    res = run_bass_kernel_spmd(nc, in_maps, list(range(N_CORES)))
    outs = [np.asarray(r["out"]) for r in res.results]
    return np.concatenate(outs, axis=0).reshape(B, C, H, W).astype(np.float32)


# revision 16
# speedup vs baseline: 1.1786x; 1.0657x over previous
"""ChannelGuidedAttn Trainium2 kernel.

Reference computation (per batch b):
    q  = x_pre[b]      reshaped (C, N),  C=512, N=H*W=4096
    kv = x_training[b] reshaped (C, N)
    energy[c,d] = <q[c,:], kv[d,:]>                      (C x C)
    att = softmax(max_d(energy) - energy, axis=-1)       == softmax(-energy)
        = exp(min_d(energy) - energy) / sum
    out = att @ kv  -> (C, H, W);  final softmax over W

Sharding: data-parallel over batch B=16 across 8 cores (2 batches/core).

Design (v2): all transposes/casts are done on the HOST during sharding; the
device receives three pre-laid-out fp16 tensors per batch:
  - ktT[p, j, d] = kv[d, j*128+p]   (gemm1 moving operand, n on partitions)
  - qT [ct, p, j, c] = q[ct*128+c, j*128+p]  (gemm1 stationary, per c-tile)
  - kn [p, dt, n] = kv[dt*128+p, n] (gemm2 moving operand, natural layout)
Each layout is partition-major so every load DMA moves 8KB-contiguous
descriptor payloads (full bus efficiency, no 2x small-element penalty).

Device program per batch (software-pipelined over c-tiles):
  g1(ct):  energy = qT(ct)^T @ ktT  (32 fp16 matmuls into one PSUM bank)
           min-reduce (DVE), att16 = exp(min - E) with sum accum (ACT),
           normalize att16 (DVE), attT via small DMA xbar transpose.
  g2(ct):  out = attT^T @ kn  (8 x 4 fp16 matmuls), exp (ACT, fp16 out),
           per-W-segment sums (DVE), reciprocal (DVE), normalize
           (alternating DVE/Pool), fp16 store per half-c-tile.
PE stream order is g1(0), g1(1), g2(0), g1(2), g2(1), ... so the in-order PE
never stalls waiting for the attT DMA round trip. Next batch's loads are
emitted between the current batch's tasks so the (serialized) DMA engines
prefetch ktT/kn/qT behind the compute.

gemm1 runs in plain fp16 (host-rounded inputs, fp32 PSUM accumulation):
measured absmax rel err ~1.2e-2 against the f64 reference (gate 2e-2).
G1_MODE="f16q" adds a q-residual pass (err ~7e-3) at +27us PE if needed.
"""

import sys

import numpy as np

for _p in ("/opt/trn_rl_repo", "/root/.axon_site/_ro/trn_rl_repo"):
    if _p not in sys.path:
        sys.path.append(_p)

B = 16
N_CORES = 8
B_PER_CORE = B // N_CORES
C = 512
H = 64
W = 64
N = H * W
CT = C // 128  # 4 c-tiles / d-tiles
NJ = N // 128  # 32 n-chunks of 128

G1_MODE = "f16"  # "f16" (plain fp16) | "f16q" (q hi/lo split, 2-pass)


def build_program(g1_mode=None):
    from contextlib import ExitStack

    import concourse.mybir as mybir
    import concourse.tile as tile
    from concourse import bacc

    if g1_mode is None:
        g1_mode = G1_MODE
    assert g1_mode in ("f16", "f16q")
    q_split = g1_mode == "f16q"

    f32 = mybir.dt.float32
    f16 = mybir.dt.float16
    Alu = mybir.AluOpType
    Act = mybir.ActivationFunctionType
    Axis = mybir.AxisListType

    nc = bacc.Bacc()
    # Host-prepared layouts (see module docstring).
    ktT = nc.declare_dram_parameter("ktT", [B_PER_CORE, 128, NJ, C], f16, isOutput=False)
    qT = nc.declare_dram_parameter("qT", [B_PER_CORE, CT, 128, NJ, 128], f16, isOutput=False)
    if q_split:
        qlT = nc.declare_dram_parameter(
            "qlT", [B_PER_CORE, CT, 128, NJ, 128], f16, isOutput=False
        )
    kn = nc.declare_dram_parameter("kn", [B_PER_CORE, 128, CT, N], f16, isOutput=False)
    out = nc.declare_dram_parameter("out", [B_PER_CORE, C, N], f16, isOutput=True)

    with tile.TileContext(nc) as tc, ExitStack() as ctx:
        ktp = ctx.enter_context(tc.tile_pool(name="ktp", bufs=2))
        knp = ctx.enter_context(tc.tile_pool(name="knp", bufs=2))
        qtp = ctx.enter_context(tc.tile_pool(name="qtp", bufs=5 + 3 * q_split))
        attp = ctx.enter_context(tc.tile_pool(name="attp", bufs=2))
        ostp = ctx.enter_context(tc.tile_pool(name="ostp", bufs=5))
        small = ctx.enter_context(tc.tile_pool(name="small", bufs=4))
        ps_e = ctx.enter_context(tc.tile_pool(name="ps_e", bufs=3, space="PSUM"))
        ps_o = ctx.enter_context(tc.tile_pool(name="ps_o", bufs=4, space="PSUM"))

        # Per-batch SBUF tiles, created/rotated on demand.
        kt_sb = {}
        kn_sb = {}
        qt_sb = {}
        ql_sb = {}
        att_16 = {}
        att_T = {}

        def emit_ktT_chunks(b, chunks):
            # 8 chunks of 4 j's each (~1.6us apiece on the DMA queue)
            if b not in kt_sb:
                kt_sb[b] = ktp.tile([128, NJ, C], f16, tag="ktT", name=f"ktT_{b}")
            for g in chunks:
                js = slice(g * (NJ // 8), (g + 1) * (NJ // 8))
                nc.sync.dma_start(out=kt_sb[b][:, js, :], in_=ktT[b, :, js, :])

        def emit_kn_chunks(b, chunks):
            if b not in kn_sb:
                kn_sb[b] = knp.tile([128, CT, N], f16, tag="kn", name=f"kn_{b}")
            for dt in chunks:
                nc.sync.dma_start(out=kn_sb[b][:, dt, :], in_=kn[b, :, dt, :])

        def emit_qT_load(b, ct, halves=1):
            qt_sb[(b, ct)] = qtp.tile([128, NJ, 128], f16, tag="qT", name=f"qT_{b}_{ct}")
            for hh in range(halves):
                js = slice(hh * (NJ // halves), (hh + 1) * (NJ // halves))
                nc.sync.dma_start(out=qt_sb[(b, ct)][:, js, :], in_=qT[b, ct, :, js, :])
            if q_split:
                ql_sb[(b, ct)] = qtp.tile(
                    [128, NJ, 128], f16, tag="qlT", name=f"qlT_{b}_{ct}"
                )
                nc.sync.dma_start(out=ql_sb[(b, ct)], in_=qlT[b, ct])

        def emit_g1(b, ct):
            # energy for this c-tile, then attention row softmax + transpose
            e_ps = ps_e.tile([128, C], f32, tag="ps_e")
            qt = qt_sb.pop((b, ct))
            ql = ql_sb.pop((b, ct), None)
            kt = kt_sb[b]
            for j in range(NJ):
                last = j == NJ - 1
                nc.tensor.matmul(
                    e_ps, qt[:, j, :], kt[:, j, :],
                    start=(j == 0), stop=(last and not q_split),
                )
                if q_split:
                    nc.tensor.matmul(
                        e_ps, ql[:, j, :], kt[:, j, :], start=False, stop=last
                    )
            min_t = small.tile([128, 1], f32, tag="min")
            nc.vector.tensor_reduce(min_t, e_ps, axis=Axis.X, op=Alu.min)
            att16 = attp.tile([128, C], f16, tag="att16")
            den = small.tile([128, 1], f32, tag="den")
            nc.scalar.activation(
                out=att16, in_=e_ps, func=Act.Exp, bias=min_t, scale=-1.0,
                accum_out=den,
            )
            rden = small.tile([128, 1], f32, tag="rden")
            nc.vector.reciprocal(rden, den)
            nc.vector.tensor_scalar_mul(att16, att16, rden)
            att_16[(b, ct)] = att16

        def emit_attT(b, ct):
            # placed explicitly in the DMA queue: late enough that its sem
            # wait (att16 ready) never head-of-line-blocks loads behind it
            att_T[(b, ct)] = attp.tile(
                [128, CT, 128], f16, tag="attT", name=f"attT_{b}_{ct}"
            )
            nc.sync.dma_start_transpose(att_T[(b, ct)], att_16.pop((b, ct)))

        def emit_g2(b, ct, tail=False, last=False):
            # out rows for this c-tile: gemm2 + final softmax over W + store.
            # tail: the final two tasks — DVE is the scarce engine there, so
            # push most normalizes to Pool (DVE keeps nj 5,7 for low latency).
            attT = att_T.pop((b, ct))
            knb = kn_sb[b]
            for h in range(2):
                o16 = ostp.tile([128, 2048], f16, tag="ost")  # 4 nj chunks of 512
                for k in range(4):
                    nj = h * 4 + k
                    o_ps = ps_o.tile([128, 512], f32, tag="ps_o")
                    for dt in range(CT):
                        nc.tensor.matmul(
                            o_ps,
                            attT[:, dt, :],
                            knb[:, dt, nj * 512 : (nj + 1) * 512],
                            start=(dt == 0),
                            stop=(dt == CT - 1),
                        )
                    o16v = o16[:, k * 512 : (k + 1) * 512].rearrange(
                        "p (s w) -> p s w", w=W
                    )
                    nc.scalar.activation(
                        out=o16v,
                        in_=o_ps.rearrange("p (s w) -> p s w", w=W),
                        func=Act.Exp,
                    )
                    ssum = small.tile([128, 512 // W], f32, tag="ssum")
                    nc.vector.tensor_reduce(ssum, o16v, axis=Axis.X, op=Alu.add)
                    rsum = small.tile([128, 512 // W], f32, tag="rsum")
                    nc.vector.reciprocal(rsum, ssum)
                    if tail:
                        on_pool = nj not in (5, 7)
                    else:
                        on_pool = nj % 2 == 0
                    eng = nc.gpsimd if on_pool else nc.vector
                    eng.tensor_tensor(
                        out=o16v,
                        in0=o16v,
                        in1=rsum[:, :, None].to_broadcast(o16v.shape),
                        op=Alu.mult,
                    )
                    if last:
                        # per-nj stores to shorten the kernel tail
                        nc.sync.dma_start(
                            out=out[
                                b,
                                ct * 128 : (ct + 1) * 128,
                                nj * 512 : (nj + 1) * 512,
                            ],
                            in_=o16[:, k * 512 : (k + 1) * 512],
                        )
                if not last:
                    nc.sync.dma_start(
                        out=out[
                            b, ct * 128 : (ct + 1) * 128, h * 2048 : (h + 1) * 2048
                        ],
                        in_=o16,
                    )

        # ---- explicit software-pipelined schedule (2 batches) ----
        # DMA queue and the in-order PE stream are co-scheduled: batch 0 runs
        # all four gemm1s first (its kn/attT can't be ready earlier anyway);
        # batch 1 prefetches are slotted so no PE task ever waits on a load.
        assert B_PER_CORE == 2
        # Hand-scheduled against the cost model: PE slot sequence is
        # g1(00..03), g2(00), g2(01), g1(10), g2(02), g1(11), g2(03),
        # g1(12), g2(10), g1(13), g2(11), g2(12), g2(13); every DMA is
        # placed so it completes just before its consuming PE slot and no
        # sem-waiting DMA ever blocks a load queued behind it.
        emit_qT_load(0, 0, halves=2)
        emit_ktT_chunks(0, range(0, 4))
        emit_qT_load(0, 1)
        emit_ktT_chunks(0, range(4, 8))
        emit_qT_load(0, 2)
        emit_qT_load(0, 3)
        emit_kn_chunks(0, [0, 1, 2])
        emit_g1(0, 0)
        emit_attT(0, 0)
        emit_kn_chunks(0, [3])
        emit_g1(0, 1)
        emit_attT(0, 1)
        emit_g1(0, 2)
        emit_attT(0, 2)
        emit_qT_load(1, 0)
        emit_ktT_chunks(1, range(0, 2))
        emit_g1(0, 3)
        emit_attT(0, 3)
        emit_ktT_chunks(1, range(2, 8))
        emit_g2(0, 0)  # st00 x2
        emit_qT_load(1, 1)
        emit_kn_chunks(1, [0, 1])
        emit_g2(0, 1)  # st01 x2
        emit_g1(1, 0)
        emit_qT_load(1, 2)
        emit_kn_chunks(1, [2, 3])
        emit_attT(1, 0)
        emit_g2(0, 2)  # st02 x2
        emit_g1(1, 1)
        emit_qT_load(1, 3)
        emit_attT(1, 1)
        emit_g2(0, 3)  # st03 x2
        emit_g1(1, 2)
        emit_attT(1, 2)
        emit_g2(1, 0)  # st10 x2
        emit_g1(1, 3)
        emit_attT(1, 3)
        emit_g2(1, 1)  # st11 x2
        emit_g2(1, 2, tail=True)
        emit_g2(1, 3, tail=True, last=True)

    nc.finalize()
    return nc


def prepare_in_maps(x_training, x_pre, g1_mode=None):
    """Host-side shard + layout prep. Returns per-core input dicts."""
    if g1_mode is None:
        g1_mode = G1_MODE
    xt = np.asarray(x_training, dtype=np.float32).reshape(B, C, N)
    xp = np.asarray(x_pre, dtype=np.float32).reshape(B, C, N)
    xt16 = xt.astype(np.float16)
    xp16 = xp.astype(np.float16)

    # ktT[b, p, j, d] = kv[b, d, j*128+p]
    ktT = np.ascontiguousarray(
        xt16.reshape(B, C, NJ, 128).transpose(0, 3, 2, 1)
    )
    # qT[b, ct, p, j, c] = q[b, ct*128+c, j*128+p]
    qT = np.ascontiguousarray(
        xp16.reshape(B, CT, 128, NJ, 128).transpose(0, 1, 4, 3, 2)
    )
    # kn[b, p, dt, n] = kv[b, dt*128+p, n]
    knat = np.ascontiguousarray(xt16.reshape(B, CT, 128, N).transpose(0, 2, 1, 3))
    if g1_mode == "f16q":
        ql = (xp - xp16.astype(np.float32)).astype(np.float16)
        qlT = np.ascontiguousarray(
            ql.reshape(B, CT, 128, NJ, 128).transpose(0, 1, 4, 3, 2)
        )

    in_maps = []
    for i in range(N_CORES):
        sl = slice(i * B_PER_CORE, (i + 1) * B_PER_CORE)
        m = {"ktT": ktT[sl], "qT": qT[sl], "kn": knat[sl]}
        if g1_mode == "f16q":
            m["qlT"] = qlT[sl]
        in_maps.append(m)
    return in_maps


def kernel(x_training: np.ndarray, x_pre: np.ndarray) -> np.ndarray:
    from concourse.bass_utils import run_bass_kernel_spmd

    nc = build_program()
    in_maps = prepare_in_maps(x_training, x_pre)
    res = run_bass_kernel_spmd(nc, in_maps, list(range(N_CORES)))
    outs = [np.asarray(r["out"]) for r in res.results]
    return np.concatenate(outs, axis=0).reshape(B, C, H, W).astype(np.float32)
